# revision 39
# baseline (speedup 1.0000x reference)
"""AttentionPool Trainium2 kernel.

Problem: x[B=8, S=4096, D=768] f32; att_v[768]; att_W[768, 768].
  y = tanh(x @ W); scores = y . v; w = softmax(scores over S); out = w . x  -> [B, D]

Sharding: pure data-parallel over batch B — one batch per NeuronCore, 8 cores,
no collectives.

Per-core pipeline (batch b), per 128-row sequence tile i:
  1. HWDGE f32 load of x tile into a staging ring (full-rate, no cast);
     att_W / att_v load on the second (Activation) DGE queue.
  2. PE transpose-mode (f32r: 1.5 cyc/row): x_tile -> xT psum
  3. copy-cast psum f32 -> SBUF fp8(e4m3) xT; split DVE (chunks 0-3) /
     ACT (chunks 4-5) to balance the per-tile vector-engine load
  4. PE: y = xT.T @ (16*W) fp8 DoubleRow, psum f32; 3 k-pairs x {512, 256}
  5. ACT: t = tanh(y_psum / 16) -> bf16
  6. DVE: scores[:, i%4] = sum_e t*v  (scalar_tensor_tensor accum_out)
  7. ACT, per 4 tiles: u = exp(scores) -> f32r (no max-subtraction needed:
     |scores| < ~0.5), accum_out -> Z partial column
  8. PE, per 4 tiles, deferred: p += u_i.T @ x_stage_i — f32r M=1
     matmuls (1 cyc/row) accumulating into one psum row over all tiles
Host: out = p / Z  (Z = sum of the per-partition exp accums).

The y-chain (emit_back) runs 2 tiles behind the load/transpose front so
the staggered att_W chunk loads (second DGE queue) are always emitted
before the first y-matmul that reads them. Pool groups trail their tiles
by 11 iterations: they are in-order PE barriers gated on the
y->tanh->stt->exp chain, so with less slack the PE stalls every 4 tiles.
The six W casts alternate ACT/DVE so neither early queue eats them all.

PSUM budget (8 banks x 2KB): every psum pool is split at the 512-f32 bank
width; psum is per-buf bank-granular: yA(1)+yB(1)+xtA(2)+xtB(2)+ppA(1)+ppB(1)
= 8 banks. y psum is single-buffered: tanh overlaps the next tile's
transposes, which are emitted between consecutive y-groups.

Measured ~93.6-98.7us on HW (baseline 121.5us), rel err 5.96e-3 (gate 2e-2).
"""

import sys

sys.path.insert(0, "/opt/trn_rl_repo")

import numpy as np

import concourse.bass as bass
import concourse.mybir as mybir
import concourse.tile as tile
from concourse.bass_utils import run_bass_kernel_spmd
from concourse.masks import make_identity

P = 128
S = 4096
D = 768
NT = S // P  # 32 sequence tiles
DJ = D // P  # 6 contraction chunks
NCORES = 8

F32 = mybir.dt.float32
F32R = mybir.dt.float32r
BF16 = mybir.dt.bfloat16
FP8 = mybir.dt.float8e4
DR = mybir.MatmulPerfMode.DoubleRow
ACTF = mybir.ActivationFunctionType


def _build(split_waits: bool = True) -> bass.Bass:
    nc = bass.Bass()
    # x declared f32r (same bits as f32) so the f32r transpose/pool
    # matmuls see f32r-typed producers end-to-end (BIR verifier rule)
    x_d = nc.declare_dram_parameter("x", [S, D], F32R, isOutput=False)
    v_d = nc.declare_dram_parameter("att_v", [D], F32, isOutput=False)
    w_d = nc.declare_dram_parameter("att_W", [D, D], F32, isOutput=False)
    p_d = nc.declare_dram_parameter("out_p", [1, D], F32, isOutput=True)
    z_d = nc.declare_dram_parameter("out_z", [P, NT // 4], F32, isOutput=True)

    with tile.TileContext(nc) as tc:
        with (
            tc.tile_pool(name="singles", bufs=1) as singles,
            tc.tile_pool(name="stage", bufs=16) as stage_pool,
            tc.tile_pool(name="xt", bufs=6) as xt_pool,
            tc.tile_pool(name="tbuf", bufs=4) as t_pool,
            tc.tile_pool(name="sc", bufs=9) as sc_pool,
            tc.tile_pool(name="ypsA", bufs=1, space="PSUM") as ypsA_pool,
            tc.tile_pool(name="ypsB", bufs=1, space="PSUM") as ypsB_pool,
            tc.tile_pool(name="xtpA", bufs=2, space="PSUM") as xtpA_pool,
            tc.tile_pool(name="xtpB", bufs=2, space="PSUM") as xtpB_pool,
            tc.tile_pool(name="ppsA", bufs=1, space="PSUM") as ppsA_pool,
            tc.tile_pool(name="ppsB", bufs=1, space="PSUM") as ppsB_pool,
        ):
            # f32r identity for the f32r transposes: gpsimd can't write
            # f32r (ISA), so build in f32 and round via a scalar-engine copy
            ident_f32 = singles.tile([P, P], F32)
            make_identity(nc, ident_f32)
            ident = singles.tile([P, P], F32R)
            nc.scalar.copy(out=ident, in_=ident_f32)
            v_f32 = singles.tile([P, D], F32)
            v_bc = singles.tile([P, D], BF16)
            w_f32 = singles.tile([P, DJ, D], F32)
            w8 = singles.tile([P, DJ, D], FP8)
            # per-group partial Z accumulators; host sums the values.
            zg = singles.tile([P, NT // 4], F32)
            # pooling accumulator psum row (partition 0), one accumulation
            # group across all 32 tiles, split at the psum bank boundary
            ppsA = ppsA_pool.tile([P, 512], F32)
            ppsB = ppsB_pool.tile([P, 256], F32)

            stage_tiles = {}
            xt_tiles = {}
            u_tiles = {}
            sc_tiles = {}

            def emit_w_chunk(j, eng=None):
                # Both DGE queues: six serialized W loads on one queue land
                # the last chunk at ~6.6us, stalling y(0)'s last k-pair.
                (eng or nc.scalar).dma_start(
                    out=w_f32[:, j, :], in_=w_d[j * P : (j + 1) * P, :]
                )
                # fp8 e4m3 W scaled by 16 to keep small entries out of
                # the subnormal range; tanh() folds the 1/16 back in.
                # All six on ACT: a DMA-gated cast at the head of the
                # in-order DVE queue would block every copyA behind it.
                nc.scalar.activation(
                    out=w8[:, j, :], in_=w_f32[:, j, :],
                    func=ACTF.Copy, scale=16.0,
                )

            def emit_params():
                nc.scalar.dma_start(
                    out=v_f32, in_=v_d[:][None, :].to_broadcast([P, D])
                )


            def emit_front(i):
                # load + f32r transpose + fp8 copy-out for tile i
                xs = stage_pool.tile([P, D], F32R, name="xs")
                nc.sync.dma_start(out=xs, in_=x_d[i * P : (i + 1) * P, :])
                stage_tiles[i] = xs
                xtA = xtpA_pool.tile([P, 512], F32, name="xtA")
                xtB = xtpB_pool.tile([P, 256], F32, name="xtB")
                for j in range(DJ):
                    dst = (
                        xtA[:, j * P : (j + 1) * P]
                        if j < 4
                        else xtB[:, (j - 4) * P : (j - 3) * P]
                    )
                    nc.tensor.transpose(
                        dst.bitcast(F32R),
                        xs[:, j * P : (j + 1) * P],
                        ident[:],
                    )
                xt8 = xt_pool.tile([P, DJ, P], FP8, name="xt8")
                nc.vector.tensor_copy(out=xt8[:, 0:4, :], in_=xtA)
                nc.scalar.copy(out=xt8[:, 4:6, :], in_=xtB)
                xt_tiles[i] = xt8

            def emit_back(i):
                # y matmuls + tanh + scores + exp for tile i
                xt8 = xt_tiles.pop(i)
                ypsA = ypsA_pool.tile([P, 512], F32, name="ypsA")
                ypsB = ypsB_pool.tile([P, 256], F32, name="ypsB")
                for jp in range(DJ // 2):
                    # fp8 DoubleRow: two 128-deep k-slabs per instruction
                    lhsT = xt8[:, 2 * jp : 2 * jp + 2, :]
                    nc.tensor.matmul(
                        ypsA,
                        lhsT=lhsT,
                        rhs=w8[:, 2 * jp : 2 * jp + 2, 0:512],
                        start=(jp == 0),
                        stop=(jp == DJ // 2 - 1),
                        perf_mode=DR,
                    )
                    nc.tensor.matmul(
                        ypsB,
                        lhsT=lhsT,
                        rhs=w8[:, 2 * jp : 2 * jp + 2, 512:D],
                        start=(jp == 0),
                        stop=(jp == DJ // 2 - 1),
                        perf_mode=DR,
                    )
                t = t_pool.tile([P, D], BF16, name="t")
                # y was computed against 16*W; tanh(y/16) undoes the scale
                nc.scalar.activation(
                    out=t[:, 0:512], in_=ypsA, func=ACTF.Tanh, scale=1.0 / 16
                )
                nc.scalar.activation(
                    out=t[:, 512:D], in_=ypsB, func=ACTF.Tanh, scale=1.0 / 16
                )
                if i % 4 == 0:
                    sc4 = sc_pool.tile([P, 4], F32, name="sc4")
                    sc_tiles[i // 4] = sc4
                sc4 = sc_tiles[i // 4]
                dve_out = t_pool.tile([P, D], FP8, name="dve_out")
                nc.vector.scalar_tensor_tensor(
                    out=dve_out,
                    in0=t,
                    scalar=1.0,
                    in1=v_bc,
                    op0=mybir.AluOpType.mult,
                    op1=mybir.AluOpType.mult,
                    accum_out=sc4[:, i % 4 : i % 4 + 1],
                )
                if i % 4 == 3:
                    g = i // 4
                    u4 = sc_pool.tile([P, 4], F32R, name="u4")
                    nc.scalar.activation(
                        out=u4, in_=sc_tiles.pop(g), func=ACTF.Exp,
                        accum_out=zg[:, g : g + 1],
                    )
                    u_tiles[g] = u4

            def emit_pool_tile(i, u):
                # f32r M=1 matmuls (1 cyc/row), one long accumulation
                # group in a single psum row across all 32 tiles
                xs = stage_tiles.pop(i)
                nc.tensor.matmul(
                    ppsA[0:1, :],
                    lhsT=u, rhs=xs[:, 0:512],
                    start=(i == 0), stop=(i == NT - 1),
                    skip_group_check=True,
                )
                nc.tensor.matmul(
                    ppsB[0:1, :],
                    lhsT=u, rhs=xs[:, 512:D],
                    start=(i == 0), stop=(i == NT - 1),
                    skip_group_check=True,
                )

            def emit_pool_group(k):
                u4 = u_tiles.pop(k // 4)
                for i in range(k - 3, k + 1):
                    emit_pool_tile(i, u4[:, i % 4 : i % 4 + 1])

            # W pair j is always emitted at least one iteration before the
            # first y-matmul that reads it (back runs 2 tiles behind front)
            emit_w_chunk(0)
            emit_w_chunk(1)
            for i in range(NT + 11):
                if i < NT:
                    emit_front(i)
                if i == 0:
                    emit_w_chunk(2, eng=nc.sync)
                    emit_w_chunk(4)
                if i == 1:
                    emit_w_chunk(3, eng=nc.sync)
                    emit_w_chunk(5)
                    emit_params()
                # pool group k trails the y-chain of its tiles by ~9
                # iterations (in-order PE barriers gated on the exp chain);
                # the last three groups are pulled in between the final
                # backs so the PE pools while it waits on the last copy-outs
                POOL_AT = {14: 3, 18: 7, 22: 11, 26: 15, 30: 19,
                           31: 23, 33: 27, 35: 31}
                if i in POOL_AT:
                    emit_pool_group(POOL_AT[i])
                if i == 2:
                    nc.vector.tensor_copy(out=v_bc, in_=v_f32)
                if 2 <= i <= NT + 1:
                    emit_back(i - 2)

            # write out the unnormalized p row and the Z partials; the host
            # divides p by sum(out_z)
            p_sb = singles.tile([1, D], F32)
            nc.scalar.copy(out=p_sb[:, 0:512], in_=ppsA[0:1, :])
            nc.scalar.copy(out=p_sb[:, 512:D], in_=ppsB[0:1, :])
            nc.sync.dma_start(out=p_d[:, :], in_=p_sb)
            nc.sync.dma_start(out=z_d[:, :], in_=zg)

    if split_waits:
        _split_excess_waits(nc)
    return nc


def _split_excess_waits(nc: bass.Bass) -> None:
    """Walrus accepts a single HW sync-wait per instruction (EventSemaphore
    excepted). Tile can attach more (data dep + DMA-lane reuse). Move all but
    one wait onto InstEventSemaphore(s) inserted just before, on the same
    engine — the sequencer executes waits in order, so semantics are
    unchanged."""
    fn = nc.m.functions[0]
    for blk in fn.blocks:
        insts = blk.instructions
        new_insts = []
        for inst in insts:
            si = inst.sync_info
            if (
                not isinstance(inst, mybir.InstEventSemaphore)
                and si is not None
                and len(si.on_wait) > 1
            ):
                waits = list(si.on_wait)
                for w in waits[:-1]:
                    ev = mybir.InstEventSemaphore(
                        name=nc.get_next_instruction_name(), ins=[], outs=[]
                    )
                    ev.engine = inst.engine
                    ev.sync_info = mybir.SyncInfo(on_wait=[w], on_update=[])
                    new_insts.append(ev)
                inst.sync_info = mybir.SyncInfo(
                    on_wait=waits[-1:], on_update=list(si.on_update)
                )
            new_insts.append(inst)
        blk.instructions = new_insts


_CACHE: dict = {}
LAST_RESULT = None


def _get_nc() -> bass.Bass:
    if "nc" not in _CACHE:
        _CACHE["nc"] = _build()
    return _CACHE["nc"]


def kernel(x: np.ndarray, att_v: np.ndarray, att_W: np.ndarray) -> np.ndarray:
    global LAST_RESULT
    assert x.shape == (NCORES, S, D), x.shape
    nc = _get_nc()
    in_maps = [
        {
            "x": np.ascontiguousarray(x[b], dtype=np.float32),
            "att_v": np.ascontiguousarray(att_v, dtype=np.float32),
            "att_W": np.ascontiguousarray(att_W, dtype=np.float32),
        }
        for b in range(NCORES)
    ]
    res = run_bass_kernel_spmd(nc, in_maps, core_ids=list(range(NCORES)))
    LAST_RESULT = res
    outs = []
    for b in range(NCORES):
        p = res.results[b]["out_p"][0].astype(np.float64)
        z = res.results[b]["out_z"].sum(dtype=np.float64)
        outs.append(p / z)
    return np.stack(outs).astype(np.float32)


# revision 40
# speedup vs baseline: 1.1065x; 1.1065x over previous
"""AttentionPool Trainium2 kernel.

Problem: x[B=8, S=4096, D=768] f32; att_v[768]; att_W[768, 768].
  y = tanh(x @ W); scores = y . v; w = softmax(scores over S); out = w . x  -> [B, D]

Sharding: pure data-parallel over batch B — one batch per NeuronCore, 8 cores,
no collectives.

Per-core pipeline (batch b), per 128-row sequence tile i:
  1. HWDGE f32 load of x tile into a staging ring (full-rate, no cast);
     att_W / att_v load on the second (Activation) DGE queue.
  2. PE transpose-mode (f32r: 1.5 cyc/row): x_tile -> xT psum
  3. copy-cast psum f32 -> SBUF fp8(e4m3) xT; split DVE (chunks 0-3) /
     ACT (chunks 4-5) to balance the per-tile vector-engine load
  4. PE: y = xT.T @ (16*W) fp8 DoubleRow, psum f32; 3 k-pairs x {512, 256}
  5. ACT: t = tanh(y_psum / 16) -> bf16
  6. DVE: scores[:, i%4] = sum_e t*v  (scalar_tensor_tensor accum_out)
  7. ACT, per 4 tiles: u = exp(scores) -> f32r (no max-subtraction needed:
     |scores| < ~0.5), accum_out -> Z partial column
  8. PE, per 4 tiles, deferred: p += u_i.T @ x_stage_i — f32r M=1
     matmuls (1 cyc/row) accumulating into one psum row over all tiles
Host: out = p / Z  (Z = sum of the per-partition exp accums).

The y-chain (emit_back) runs 2 tiles behind the load/transpose front so
the staggered att_W chunk loads (second DGE queue) are always emitted
before the first y-matmul that reads them. Pool groups trail their tiles
by 11 iterations: they are in-order PE barriers gated on the
y->tanh->stt->exp chain, so with less slack the PE stalls every 4 tiles.
The six W casts alternate ACT/DVE so neither early queue eats them all.

PSUM budget (8 banks x 2KB): every psum pool is split at the 512-f32 bank
width; psum is per-buf bank-granular: yA(1)+yB(1)+xtA(2)+xtB(2)+ppA(1)+ppB(1)
= 8 banks. y psum is single-buffered: tanh overlaps the next tile's
transposes, which are emitted between consecutive y-groups.

Measured ~93.6-98.7us on HW (baseline 121.5us), rel err 5.96e-3 (gate 2e-2).
"""

import sys

sys.path.insert(0, "/opt/trn_rl_repo")

import numpy as np

import concourse.bass as bass
import concourse.mybir as mybir
import concourse.tile as tile
from concourse.bass_utils import run_bass_kernel_spmd
from concourse.masks import make_identity

P = 128
S = 4096
D = 768
NT = S // P  # 32 sequence tiles
DJ = D // P  # 6 contraction chunks
NCORES = 8

F32 = mybir.dt.float32
F32R = mybir.dt.float32r
BF16 = mybir.dt.bfloat16
FP8 = mybir.dt.float8e4
DR = mybir.MatmulPerfMode.DoubleRow
ACTF = mybir.ActivationFunctionType


def _build(split_waits: bool = True) -> bass.Bass:
    nc = bass.Bass()
    # x declared f32r (same bits as f32) so the f32r transpose/pool
    # matmuls see f32r-typed producers end-to-end (BIR verifier rule)
    x_d = nc.declare_dram_parameter("x", [S, D], F32R, isOutput=False)
    v_d = nc.declare_dram_parameter("att_v", [D], F32, isOutput=False)
    w_d = nc.declare_dram_parameter("att_W", [D, D], F32, isOutput=False)
    p_d = nc.declare_dram_parameter("out_p", [1, D], F32, isOutput=True)
    z_d = nc.declare_dram_parameter("out_z", [P, NT // 4], F32, isOutput=True)

    with tile.TileContext(nc) as tc:
        with (
            tc.tile_pool(name="singles", bufs=1) as singles,
            tc.tile_pool(name="stage", bufs=16) as stage_pool,
            tc.tile_pool(name="xt", bufs=6) as xt_pool,
            tc.tile_pool(name="tbuf", bufs=4) as t_pool,
            tc.tile_pool(name="sc", bufs=9) as sc_pool,
            tc.tile_pool(name="ypsA", bufs=1, space="PSUM") as ypsA_pool,
            tc.tile_pool(name="ypsB", bufs=1, space="PSUM") as ypsB_pool,
            tc.tile_pool(name="xtpA", bufs=2, space="PSUM") as xtpA_pool,
            tc.tile_pool(name="xtpB", bufs=2, space="PSUM") as xtpB_pool,
            tc.tile_pool(name="ppsA", bufs=1, space="PSUM") as ppsA_pool,
            tc.tile_pool(name="ppsB", bufs=1, space="PSUM") as ppsB_pool,
        ):
            # f32r identity for the f32r transposes: gpsimd can't write
            # f32r (ISA), so build in f32 and round via a scalar-engine copy
            ident_f32 = singles.tile([P, P], F32)
            make_identity(nc, ident_f32)
            ident = singles.tile([P, P], F32R)
            nc.scalar.copy(out=ident, in_=ident_f32)
            v_f32 = singles.tile([P, D], F32)
            v_bc = singles.tile([P, D], BF16)
            w_f32 = singles.tile([P, DJ, D], F32)
            w8 = singles.tile([P, DJ, D], FP8)
            # per-group partial Z accumulators; host sums the values.
            zg = singles.tile([P, NT // 4], F32)
            # pooling accumulator psum row (partition 0), one accumulation
            # group across all 32 tiles, split at the psum bank boundary
            ppsA = ppsA_pool.tile([P, 512], F32)
            ppsB = ppsB_pool.tile([P, 256], F32)

            stage_tiles = {}
            xt_tiles = {}
            u_tiles = {}
            sc_tiles = {}

            def emit_w_chunk(j, eng=None):
                # Both DGE queues: six serialized W loads on one queue land
                # the last chunk at ~6.6us, stalling y(0)'s last k-pair.
                (eng or nc.scalar).dma_start(
                    out=w_f32[:, j, :], in_=w_d[j * P : (j + 1) * P, :]
                )
                # fp8 e4m3 W scaled by 16 to keep small entries out of
                # the subnormal range; tanh() folds the 1/16 back in.
                # All six on ACT: a DMA-gated cast at the head of the
                # in-order DVE queue would block every copyA behind it.
                nc.scalar.activation(
                    out=w8[:, j, :], in_=w_f32[:, j, :],
                    func=ACTF.Copy, scale=16.0,
                )

            def emit_params():
                nc.scalar.dma_start(
                    out=v_f32, in_=v_d[:][None, :].to_broadcast([P, D])
                )


            def emit_front(i):
                # load + f32r transpose + fp8 copy-out for tile i
                xs = stage_pool.tile([P, D], F32R, name="xs")
                nc.sync.dma_start(out=xs, in_=x_d[i * P : (i + 1) * P, :])
                stage_tiles[i] = xs
                xtA = xtpA_pool.tile([P, 512], F32, name="xtA")
                xtB = xtpB_pool.tile([P, 256], F32, name="xtB")
                for j in range(DJ):
                    dst = (
                        xtA[:, j * P : (j + 1) * P]
                        if j < 4
                        else xtB[:, (j - 4) * P : (j - 3) * P]
                    )
                    nc.tensor.transpose(
                        dst.bitcast(F32R),
                        xs[:, j * P : (j + 1) * P],
                        ident[:],
                    )
                xt8 = xt_pool.tile([P, DJ, P], FP8, name="xt8")
                nc.vector.tensor_copy(out=xt8[:, 0:4, :], in_=xtA)
                nc.scalar.copy(out=xt8[:, 4:6, :], in_=xtB)
                xt_tiles[i] = xt8

            def emit_back(i):
                # y matmuls + tanh + scores + exp for tile i
                xt8 = xt_tiles.pop(i)
                ypsA = ypsA_pool.tile([P, 512], F32, name="ypsA")
                ypsB = ypsB_pool.tile([P, 256], F32, name="ypsB")
                for jp in range(DJ // 2):
                    # fp8 DoubleRow: two 128-deep k-slabs per instruction
                    lhsT = xt8[:, 2 * jp : 2 * jp + 2, :]
                    nc.tensor.matmul(
                        ypsA,
                        lhsT=lhsT,
                        rhs=w8[:, 2 * jp : 2 * jp + 2, 0:512],
                        start=(jp == 0),
                        stop=(jp == DJ // 2 - 1),
                        perf_mode=DR,
                    )
                    nc.tensor.matmul(
                        ypsB,
                        lhsT=lhsT,
                        rhs=w8[:, 2 * jp : 2 * jp + 2, 512:D],
                        start=(jp == 0),
                        stop=(jp == DJ // 2 - 1),
                        perf_mode=DR,
                    )
                t = t_pool.tile([P, D], BF16, name="t")
                # y was computed against 16*W; tanh(y/16) undoes the scale
                nc.scalar.activation(
                    out=t[:, 0:512], in_=ypsA, func=ACTF.Tanh, scale=1.0 / 16
                )
                nc.scalar.activation(
                    out=t[:, 512:D], in_=ypsB, func=ACTF.Tanh, scale=1.0 / 16
                )
                if i % 4 == 0:
                    sc4 = sc_pool.tile([P, 4], F32, name="sc4")
                    sc_tiles[i // 4] = sc4
                sc4 = sc_tiles[i // 4]
                dve_out = t_pool.tile([P, D], FP8, name="dve_out")
                nc.vector.scalar_tensor_tensor(
                    out=dve_out,
                    in0=t,
                    scalar=1.0,
                    in1=v_bc,
                    op0=mybir.AluOpType.mult,
                    op1=mybir.AluOpType.mult,
                    accum_out=sc4[:, i % 4 : i % 4 + 1],
                )
                if i % 4 == 3:
                    g = i // 4
                    u4 = sc_pool.tile([P, 4], F32R, name="u4")
                    nc.scalar.activation(
                        out=u4, in_=sc_tiles.pop(g), func=ACTF.Exp,
                        accum_out=zg[:, g : g + 1],
                    )
                    u_tiles[g] = u4

            def emit_pool_tile(i, u):
                # f32r M=1 matmuls (1 cyc/row), one long accumulation
                # group in a single psum row across all 32 tiles
                xs = stage_tiles.pop(i)
                nc.tensor.matmul(
                    ppsA[0:1, :],
                    lhsT=u, rhs=xs[:, 0:512],
                    start=(i == 0), stop=(i == NT - 1),
                    skip_group_check=True,
                )
                nc.tensor.matmul(
                    ppsB[0:1, :],
                    lhsT=u, rhs=xs[:, 512:D],
                    start=(i == 0), stop=(i == NT - 1),
                    skip_group_check=True,
                )

            def emit_pool_group(k):
                u4 = u_tiles.pop(k // 4)
                for i in range(k - 3, k + 1):
                    emit_pool_tile(i, u4[:, i % 4 : i % 4 + 1])

            # W pair j is always emitted at least one iteration before the
            # first y-matmul that reads it (back runs 2 tiles behind front)
            emit_w_chunk(0)
            emit_w_chunk(1)
            for i in range(NT + 11):
                if i < NT:
                    emit_front(i)
                if i == 0:
                    emit_w_chunk(2, eng=nc.sync)
                    emit_w_chunk(4)
                if i == 1:
                    emit_w_chunk(3, eng=nc.sync)
                    emit_w_chunk(5)
                    emit_params()
                # pool group k trails the y-chain of its tiles by ~9
                # iterations: pool groups are in-order PE barriers, so they
                # must never lead the ACT/DVE exp chain they depend on
                k = i - 11
                if k >= 3 and k % 4 == 3:
                    emit_pool_group(k)
                if i == 2:
                    nc.vector.tensor_copy(out=v_bc, in_=v_f32)
                if 2 <= i <= NT + 1:
                    emit_back(i - 2)

            # write out the unnormalized p row and the Z partials; the host
            # divides p by sum(out_z)
            p_sb = singles.tile([1, D], F32)
            nc.scalar.copy(out=p_sb[:, 0:512], in_=ppsA[0:1, :])
            nc.scalar.copy(out=p_sb[:, 512:D], in_=ppsB[0:1, :])
            nc.sync.dma_start(out=p_d[:, :], in_=p_sb)
            nc.sync.dma_start(out=z_d[:, :], in_=zg)

    if split_waits:
        _split_excess_waits(nc)
    return nc


def _split_excess_waits(nc: bass.Bass) -> None:
    """Walrus accepts a single HW sync-wait per instruction (EventSemaphore
    excepted). Tile can attach more (data dep + DMA-lane reuse). Move all but
    one wait onto InstEventSemaphore(s) inserted just before, on the same
    engine — the sequencer executes waits in order, so semantics are
    unchanged."""
    fn = nc.m.functions[0]
    for blk in fn.blocks:
        insts = blk.instructions
        new_insts = []
        for inst in insts:
            si = inst.sync_info
            if (
                not isinstance(inst, mybir.InstEventSemaphore)
                and si is not None
                and len(si.on_wait) > 1
            ):
                waits = list(si.on_wait)
                for w in waits[:-1]:
                    ev = mybir.InstEventSemaphore(
                        name=nc.get_next_instruction_name(), ins=[], outs=[]
                    )
                    ev.engine = inst.engine
                    ev.sync_info = mybir.SyncInfo(on_wait=[w], on_update=[])
                    new_insts.append(ev)
                inst.sync_info = mybir.SyncInfo(
                    on_wait=waits[-1:], on_update=list(si.on_update)
                )
            new_insts.append(inst)
        blk.instructions = new_insts


_CACHE: dict = {}
LAST_RESULT = None


def _get_nc() -> bass.Bass:
    if "nc" not in _CACHE:
        _CACHE["nc"] = _build()
    return _CACHE["nc"]


def kernel(x: np.ndarray, att_v: np.ndarray, att_W: np.ndarray) -> np.ndarray:
    global LAST_RESULT
    assert x.shape == (NCORES, S, D), x.shape
    nc = _get_nc()
    in_maps = [
        {
            "x": np.ascontiguousarray(x[b], dtype=np.float32),
            "att_v": np.ascontiguousarray(att_v, dtype=np.float32),
            "att_W": np.ascontiguousarray(att_W, dtype=np.float32),
        }
        for b in range(NCORES)
    ]
    res = run_bass_kernel_spmd(nc, in_maps, core_ids=list(range(NCORES)))
    LAST_RESULT = res
    outs = []
    for b in range(NCORES):
        p = res.results[b]["out_p"][0].astype(np.float64)
        z = res.results[b]["out_z"].sum(dtype=np.float64)
        outs.append(p / z)
    return np.stack(outs).astype(np.float32)


# revision 41
# speedup vs baseline: 1.1223x; 1.0143x over previous
"""AttentionPool Trainium2 kernel.

Problem: x[B=8, S=4096, D=768] f32; att_v[768]; att_W[768, 768].
  y = tanh(x @ W); scores = y . v; w = softmax(scores over S); out = w . x  -> [B, D]

Sharding: pure data-parallel over batch B — one batch per NeuronCore, 8 cores,
no collectives.

Per-core pipeline (batch b), per 128-row sequence tile i:
  1. HWDGE f32 load of x tile into a staging ring (full-rate, no cast);
     att_W / att_v load on the second (Activation) DGE queue.
  2. PE transpose-mode (f32r: 1.5 cyc/row): x_tile -> xT psum
  3. copy-cast psum f32 -> SBUF fp8(e4m3) xT; split DVE (chunks 0-3) /
     ACT (chunks 4-5) to balance the per-tile vector-engine load
  4. PE: y = xT.T @ (16*W) fp8 DoubleRow, psum f32; 3 k-pairs x {512, 256}
  5. ACT: t = tanh(y_psum / 16) -> bf16
  6. DVE: scores[:, i%4] = sum_e t*v  (scalar_tensor_tensor accum_out)
  7. ACT, per 4 tiles: u = exp(scores) -> f32r (no max-subtraction needed:
     |scores| < ~0.5), accum_out -> Z partial column
  8. PE, per 4 tiles, deferred: p += u_i.T @ x_stage_i — f32r M=1
     matmuls (1 cyc/row) accumulating into one psum row over all tiles
Host: out = p / Z  (Z = sum of the per-partition exp accums).

The y-chain (emit_back) runs 2 tiles behind the load/transpose front so
the staggered att_W chunk loads (second DGE queue) are always emitted
before the first y-matmul that reads them. Pool groups trail their tiles
by 11 iterations: they are in-order PE barriers gated on the
y->tanh->stt->exp chain, so with less slack the PE stalls every 4 tiles.
The six W casts alternate ACT/DVE so neither early queue eats them all.

PSUM budget (8 banks x 2KB): every psum pool is split at the 512-f32 bank
width; psum is per-buf bank-granular: yA(1)+yB(1)+xtA(2)+xtB(2)+ppA(1)+ppB(1)
= 8 banks. y psum is single-buffered: tanh overlaps the next tile's
transposes, which are emitted between consecutive y-groups.

Measured ~93.6-98.7us on HW (baseline 121.5us), rel err 5.96e-3 (gate 2e-2).
"""

import sys

sys.path.insert(0, "/opt/trn_rl_repo")

import numpy as np

import concourse.bass as bass
import concourse.mybir as mybir
import concourse.tile as tile
from concourse.bass_utils import run_bass_kernel_spmd
from concourse.masks import make_identity

P = 128
S = 4096
D = 768
NT = S // P  # 32 sequence tiles
DJ = D // P  # 6 contraction chunks
NCORES = 8

F32 = mybir.dt.float32
F32R = mybir.dt.float32r
BF16 = mybir.dt.bfloat16
FP8 = mybir.dt.float8e4
DR = mybir.MatmulPerfMode.DoubleRow
ACTF = mybir.ActivationFunctionType


def _build(split_waits: bool = True) -> bass.Bass:
    nc = bass.Bass()
    # x declared f32r (same bits as f32) so the f32r transpose/pool
    # matmuls see f32r-typed producers end-to-end (BIR verifier rule)
    x_d = nc.declare_dram_parameter("x", [S, D], F32R, isOutput=False)
    v_d = nc.declare_dram_parameter("att_v", [D], F32, isOutput=False)
    w_d = nc.declare_dram_parameter("att_W", [D, D], F32, isOutput=False)
    p_d = nc.declare_dram_parameter("out_p", [1, D], F32, isOutput=True)
    z_d = nc.declare_dram_parameter("out_z", [P, NT // 4], F32, isOutput=True)

    with tile.TileContext(nc) as tc:
        with (
            tc.tile_pool(name="singles", bufs=1) as singles,
            tc.tile_pool(name="stage", bufs=16) as stage_pool,
            tc.tile_pool(name="xt", bufs=6) as xt_pool,
            tc.tile_pool(name="tbuf", bufs=4) as t_pool,
            tc.tile_pool(name="sc", bufs=9) as sc_pool,
            tc.tile_pool(name="ypsA", bufs=1, space="PSUM") as ypsA_pool,
            tc.tile_pool(name="ypsB", bufs=1, space="PSUM") as ypsB_pool,
            tc.tile_pool(name="xtpA", bufs=2, space="PSUM") as xtpA_pool,
            tc.tile_pool(name="xtpB", bufs=2, space="PSUM") as xtpB_pool,
            tc.tile_pool(name="ppsA", bufs=1, space="PSUM") as ppsA_pool,
            tc.tile_pool(name="ppsB", bufs=1, space="PSUM") as ppsB_pool,
        ):
            # f32r identity for the f32r transposes: gpsimd can't write
            # f32r (ISA), so build in f32 and round via a scalar-engine copy
            ident_f32 = singles.tile([P, P], F32)
            make_identity(nc, ident_f32)
            ident = singles.tile([P, P], F32R)
            nc.scalar.copy(out=ident, in_=ident_f32)
            v_f32 = singles.tile([P, D], F32)
            v_bc = singles.tile([P, D], BF16)
            w_f32 = singles.tile([P, DJ, D], F32)
            w8 = singles.tile([P, DJ, D], FP8)
            # per-group partial Z accumulators; host sums the values.
            zg = singles.tile([P, NT // 4], F32)
            # pooling accumulator psum row (partition 0), one accumulation
            # group across all 32 tiles, split at the psum bank boundary
            ppsA = ppsA_pool.tile([P, 512], F32)
            ppsB = ppsB_pool.tile([P, 256], F32)

            stage_tiles = {}
            xt_tiles = {}
            u_tiles = {}
            sc_tiles = {}

            def emit_w_chunk(j, eng=None):
                # Both DGE queues: six serialized W loads on one queue land
                # the last chunk at ~6.6us, stalling y(0)'s last k-pair.
                (eng or nc.scalar).dma_start(
                    out=w_f32[:, j, :], in_=w_d[j * P : (j + 1) * P, :]
                )
                # fp8 e4m3 W scaled by 16 to keep small entries out of
                # the subnormal range; tanh() folds the 1/16 back in.
                # All six on ACT: a DMA-gated cast at the head of the
                # in-order DVE queue would block every copyA behind it.
                nc.scalar.activation(
                    out=w8[:, j, :], in_=w_f32[:, j, :],
                    func=ACTF.Copy, scale=16.0,
                )

            def emit_params():
                nc.scalar.dma_start(
                    out=v_f32, in_=v_d[:][None, :].to_broadcast([P, D])
                )


            def emit_front(i):
                # load + f32r transpose + fp8 copy-out for tile i
                xs = stage_pool.tile([P, D], F32R, name="xs")
                nc.sync.dma_start(out=xs, in_=x_d[i * P : (i + 1) * P, :])
                stage_tiles[i] = xs
                xtA = xtpA_pool.tile([P, 512], F32, name="xtA")
                xtB = xtpB_pool.tile([P, 256], F32, name="xtB")
                for j in range(DJ):
                    dst = (
                        xtA[:, j * P : (j + 1) * P]
                        if j < 4
                        else xtB[:, (j - 4) * P : (j - 3) * P]
                    )
                    nc.tensor.transpose(
                        dst.bitcast(F32R),
                        xs[:, j * P : (j + 1) * P],
                        ident[:],
                    )
                xt8 = xt_pool.tile([P, DJ, P], FP8, name="xt8")
                nc.vector.tensor_copy(out=xt8[:, 0:4, :], in_=xtA)
                nc.scalar.copy(out=xt8[:, 4:6, :], in_=xtB)
                xt_tiles[i] = xt8

            def emit_back(i):
                # y matmuls + tanh + scores + exp for tile i
                xt8 = xt_tiles.pop(i)
                ypsA = ypsA_pool.tile([P, 512], F32, name="ypsA")
                ypsB = ypsB_pool.tile([P, 256], F32, name="ypsB")
                # fp8 DoubleRow: two 128-deep k-slabs per instruction.
                # All A-half matmuls first: the A accumulation stops two
                # matmuls earlier, so tanh-A (and the score chain behind
                # it) gets a head start on every tile.
                for jp in range(DJ // 2):
                    nc.tensor.matmul(
                        ypsA,
                        lhsT=xt8[:, 2 * jp : 2 * jp + 2, :],
                        rhs=w8[:, 2 * jp : 2 * jp + 2, 0:512],
                        start=(jp == 0),
                        stop=(jp == DJ // 2 - 1),
                        perf_mode=DR,
                    )
                for jp in range(DJ // 2):
                    nc.tensor.matmul(
                        ypsB,
                        lhsT=xt8[:, 2 * jp : 2 * jp + 2, :],
                        rhs=w8[:, 2 * jp : 2 * jp + 2, 512:D],
                        start=(jp == 0),
                        stop=(jp == DJ // 2 - 1),
                        perf_mode=DR,
                    )
                t = t_pool.tile([P, D], BF16, name="t")
                # y was computed against 16*W; tanh(y/16) undoes the scale
                nc.scalar.activation(
                    out=t[:, 0:512], in_=ypsA, func=ACTF.Tanh, scale=1.0 / 16
                )
                nc.scalar.activation(
                    out=t[:, 512:D], in_=ypsB, func=ACTF.Tanh, scale=1.0 / 16
                )
                if i % 4 == 0:
                    sc4 = sc_pool.tile([P, 4], F32, name="sc4")
                    sc_tiles[i // 4] = sc4
                sc4 = sc_tiles[i // 4]
                dve_out = t_pool.tile([P, D], FP8, name="dve_out")
                nc.vector.scalar_tensor_tensor(
                    out=dve_out,
                    in0=t,
                    scalar=1.0,
                    in1=v_bc,
                    op0=mybir.AluOpType.mult,
                    op1=mybir.AluOpType.mult,
                    accum_out=sc4[:, i % 4 : i % 4 + 1],
                )
                if i % 4 == 3:
                    g = i // 4
                    u4 = sc_pool.tile([P, 4], F32R, name="u4")
                    nc.scalar.activation(
                        out=u4, in_=sc_tiles.pop(g), func=ACTF.Exp,
                        accum_out=zg[:, g : g + 1],
                    )
                    u_tiles[g] = u4

            def emit_pool_tile(i, u):
                # f32r M=1 matmuls (1 cyc/row), one long accumulation
                # group in a single psum row across all 32 tiles
                xs = stage_tiles.pop(i)
                nc.tensor.matmul(
                    ppsA[0:1, :],
                    lhsT=u, rhs=xs[:, 0:512],
                    start=(i == 0), stop=(i == NT - 1),
                    skip_group_check=True,
                )
                nc.tensor.matmul(
                    ppsB[0:1, :],
                    lhsT=u, rhs=xs[:, 512:D],
                    start=(i == 0), stop=(i == NT - 1),
                    skip_group_check=True,
                )

            def emit_pool_group(k):
                u4 = u_tiles.pop(k // 4)
                for i in range(k - 3, k + 1):
                    emit_pool_tile(i, u4[:, i % 4 : i % 4 + 1])

            # W pair j is always emitted at least one iteration before the
            # first y-matmul that reads it (back runs 2 tiles behind front)
            emit_w_chunk(0)
            emit_w_chunk(1)
            for i in range(NT + 11):
                if i < NT:
                    emit_front(i)
                if i == 0:
                    emit_w_chunk(2, eng=nc.sync)
                    emit_w_chunk(4)
                if i == 1:
                    emit_w_chunk(3, eng=nc.sync)
                    emit_w_chunk(5)
                    emit_params()
                # pool group k trails the y-chain of its tiles by ~9
                # iterations: pool groups are in-order PE barriers, so they
                # must never lead the ACT/DVE exp chain they depend on
                k = i - 11
                if k >= 3 and k % 4 == 3:
                    emit_pool_group(k)
                if i == 2:
                    nc.vector.tensor_copy(out=v_bc, in_=v_f32)
                if 2 <= i <= NT + 1:
                    emit_back(i - 2)

            # write out the unnormalized p row and the Z partials; the host
            # divides p by sum(out_z)
            p_sb = singles.tile([1, D], F32)
            nc.scalar.copy(out=p_sb[:, 0:512], in_=ppsA[0:1, :])
            nc.scalar.copy(out=p_sb[:, 512:D], in_=ppsB[0:1, :])
            nc.sync.dma_start(out=p_d[:, :], in_=p_sb)
            nc.sync.dma_start(out=z_d[:, :], in_=zg)

    if split_waits:
        _split_excess_waits(nc)
    return nc


def _split_excess_waits(nc: bass.Bass) -> None:
    """Walrus accepts a single HW sync-wait per instruction (EventSemaphore
    excepted). Tile can attach more (data dep + DMA-lane reuse). Move all but
    one wait onto InstEventSemaphore(s) inserted just before, on the same
    engine — the sequencer executes waits in order, so semantics are
    unchanged."""
    fn = nc.m.functions[0]
    for blk in fn.blocks:
        insts = blk.instructions
        new_insts = []
        for inst in insts:
            si = inst.sync_info
            if (
                not isinstance(inst, mybir.InstEventSemaphore)
                and si is not None
                and len(si.on_wait) > 1
            ):
                waits = list(si.on_wait)
                for w in waits[:-1]:
                    ev = mybir.InstEventSemaphore(
                        name=nc.get_next_instruction_name(), ins=[], outs=[]
                    )
                    ev.engine = inst.engine
                    ev.sync_info = mybir.SyncInfo(on_wait=[w], on_update=[])
                    new_insts.append(ev)
                inst.sync_info = mybir.SyncInfo(
                    on_wait=waits[-1:], on_update=list(si.on_update)
                )
            new_insts.append(inst)
        blk.instructions = new_insts


_CACHE: dict = {}
LAST_RESULT = None


def _get_nc() -> bass.Bass:
    if "nc" not in _CACHE:
        _CACHE["nc"] = _build()
    return _CACHE["nc"]


def kernel(x: np.ndarray, att_v: np.ndarray, att_W: np.ndarray) -> np.ndarray:
    global LAST_RESULT
    assert x.shape == (NCORES, S, D), x.shape
    nc = _get_nc()
    in_maps = [
        {
            "x": np.ascontiguousarray(x[b], dtype=np.float32),
            "att_v": np.ascontiguousarray(att_v, dtype=np.float32),
            "att_W": np.ascontiguousarray(att_W, dtype=np.float32),
        }
        for b in range(NCORES)
    ]
    res = run_bass_kernel_spmd(nc, in_maps, core_ids=list(range(NCORES)))
    LAST_RESULT = res
    outs = []
    for b in range(NCORES):
        p = res.results[b]["out_p"][0].astype(np.float64)
        z = res.results[b]["out_z"].sum(dtype=np.float64)
        outs.append(p / z)
    return np.stack(outs).astype(np.float32)


# revision 43
# speedup vs baseline: 1.1369x; 1.0129x over previous
"""AttentionPool Trainium2 kernel.

Problem: x[B=8, S=4096, D=768] f32; att_v[768]; att_W[768, 768].
  y = tanh(x @ W); scores = y . v; w = softmax(scores over S); out = w . x  -> [B, D]

Sharding: pure data-parallel over batch B — one batch per NeuronCore, 8 cores,
no collectives.

Per-core pipeline (batch b), per 128-row sequence tile i:
  1. HWDGE f32 load of x tile into a staging ring (full-rate, no cast);
     att_W / att_v load on the second (Activation) DGE queue.
  2. PE transpose-mode (f32r: 1.5 cyc/row): x_tile -> xT psum
  3. copy-cast psum f32 -> SBUF fp8(e4m3) xT; split DVE (chunks 0-3) /
     ACT (chunks 4-5) to balance the per-tile vector-engine load
  4. PE: y = xT.T @ (16*W) fp8 DoubleRow, psum f32; 3 k-pairs x {512, 256}
  5. ACT: t = tanh(y_psum / 16) -> bf16
  6. DVE: scores[:, i%4] = sum_e t*v  (scalar_tensor_tensor accum_out)
  7. ACT, per 4 tiles: u = exp(scores) -> f32r (no max-subtraction needed:
     |scores| < ~0.5), accum_out -> Z partial column
  8. PE, per 4 tiles, deferred: p += u_i.T @ x_stage_i — f32r M=1
     matmuls (1 cyc/row) accumulating into one psum row over all tiles
Host: out = p / Z  (Z = sum of the per-partition exp accums).

The y-chain (emit_back) runs 2 tiles behind the load/transpose front so
the staggered att_W chunk loads (second DGE queue) are always emitted
before the first y-matmul that reads them. Pool groups trail their tiles
by 11 iterations: they are in-order PE barriers gated on the
y->tanh->stt->exp chain, so with less slack the PE stalls every 4 tiles.
The six W casts alternate ACT/DVE so neither early queue eats them all.

PSUM budget (8 banks x 2KB): every psum pool is split at the 512-f32 bank
width; psum is per-buf bank-granular: yA(1)+yB(1)+xtA(2)+xtB(2)+ppA(1)+ppB(1)
= 8 banks. y psum is single-buffered: tanh overlaps the next tile's
transposes, which are emitted between consecutive y-groups.

Measured ~93.6-98.7us on HW (baseline 121.5us), rel err 5.96e-3 (gate 2e-2).
"""

import sys

sys.path.insert(0, "/opt/trn_rl_repo")

import numpy as np

import concourse.bass as bass
import concourse.mybir as mybir
import concourse.tile as tile
from concourse.bass_utils import run_bass_kernel_spmd
from concourse.masks import make_identity

P = 128
S = 4096
D = 768
NT = S // P  # 32 sequence tiles
DJ = D // P  # 6 contraction chunks
NCORES = 8

F32 = mybir.dt.float32
F32R = mybir.dt.float32r
BF16 = mybir.dt.bfloat16
FP8 = mybir.dt.float8e4
DR = mybir.MatmulPerfMode.DoubleRow
ACTF = mybir.ActivationFunctionType


def _build(split_waits: bool = True) -> bass.Bass:
    nc = bass.Bass()
    # x declared f32r (same bits as f32) so the f32r transpose/pool
    # matmuls see f32r-typed producers end-to-end (BIR verifier rule)
    x_d = nc.declare_dram_parameter("x", [S, D], F32R, isOutput=False)
    v_d = nc.declare_dram_parameter("att_v", [D], F32, isOutput=False)
    w_d = nc.declare_dram_parameter("att_W", [D, D], F32, isOutput=False)
    p_d = nc.declare_dram_parameter("out_p", [1, D], F32, isOutput=True)
    z_d = nc.declare_dram_parameter("out_z", [P, NT // 4], F32, isOutput=True)

    with tile.TileContext(nc) as tc:
        with (
            tc.tile_pool(name="singles", bufs=1) as singles,
            tc.tile_pool(name="stage", bufs=16) as stage_pool,
            tc.tile_pool(name="xt", bufs=6) as xt_pool,
            tc.tile_pool(name="tbuf", bufs=4) as t_pool,
            tc.tile_pool(name="sc", bufs=9) as sc_pool,
            tc.tile_pool(name="ypsA", bufs=1, space="PSUM") as ypsA_pool,
            tc.tile_pool(name="ypsB", bufs=1, space="PSUM") as ypsB_pool,
            tc.tile_pool(name="xtpA", bufs=2, space="PSUM") as xtpA_pool,
            tc.tile_pool(name="xtpB", bufs=2, space="PSUM") as xtpB_pool,
            tc.tile_pool(name="ppsA", bufs=1, space="PSUM") as ppsA_pool,
            tc.tile_pool(name="ppsB", bufs=1, space="PSUM") as ppsB_pool,
        ):
            # f32r identity for the f32r transposes: gpsimd can't write
            # f32r (ISA), so build in f32 and round via a scalar-engine copy
            ident_f32 = singles.tile([P, P], F32)
            make_identity(nc, ident_f32)
            ident = singles.tile([P, P], F32R)
            nc.scalar.copy(out=ident, in_=ident_f32)
            v_f32 = singles.tile([P, D], F32)
            v_bc = singles.tile([P, D], BF16)
            w_f32 = singles.tile([P, DJ, D], F32)
            w8 = singles.tile([P, DJ, D], FP8)
            # per-group partial Z accumulators; host sums the values.
            zg = singles.tile([P, NT // 4], F32)
            # pooling accumulator psum row (partition 0), one accumulation
            # group across all 32 tiles, split at the psum bank boundary
            ppsA = ppsA_pool.tile([P, 512], F32)
            ppsB = ppsB_pool.tile([P, 256], F32)

            stage_tiles = {}
            xt_tiles = {}
            u_tiles = {}
            sc_tiles = {}

            def emit_w_chunk(j, eng=None):
                # Both DGE queues: six serialized W loads on one queue land
                # the last chunk at ~6.6us, stalling y(0)'s last k-pair.
                (eng or nc.scalar).dma_start(
                    out=w_f32[:, j, :], in_=w_d[j * P : (j + 1) * P, :]
                )
                # fp8 e4m3 W scaled by 16 to keep small entries out of
                # the subnormal range; tanh() folds the 1/16 back in.
                # All six on ACT: a DMA-gated cast at the head of the
                # in-order DVE queue would block every copyA behind it.
                nc.scalar.activation(
                    out=w8[:, j, :], in_=w_f32[:, j, :],
                    func=ACTF.Copy, scale=16.0,
                )

            def emit_params():
                # v DMA lands between W1 and W4 on the scalar queue (~3.3us)
                # so the v_bc copy never blocks the early DVE queue
                nc.scalar.dma_start(
                    out=v_f32, in_=v_d[:][None, :].to_broadcast([P, D])
                )


            def emit_front(i):
                # load + f32r transpose + fp8 copy-out for tile i
                xs = stage_pool.tile([P, D], F32R, name="xs")
                nc.sync.dma_start(out=xs, in_=x_d[i * P : (i + 1) * P, :])
                stage_tiles[i] = xs
                xtA = xtpA_pool.tile([P, 512], F32, name="xtA")
                xtB = xtpB_pool.tile([P, 256], F32, name="xtB")
                for j in range(DJ):
                    dst = (
                        xtA[:, j * P : (j + 1) * P]
                        if j < 4
                        else xtB[:, (j - 4) * P : (j - 3) * P]
                    )
                    nc.tensor.transpose(
                        dst.bitcast(F32R),
                        xs[:, j * P : (j + 1) * P],
                        ident[:],
                    )
                xt8 = xt_pool.tile([P, DJ, P], FP8, name="xt8")
                nc.vector.tensor_copy(out=xt8[:, 0:4, :], in_=xtA)
                nc.scalar.copy(out=xt8[:, 4:6, :], in_=xtB)
                xt_tiles[i] = xt8

            def emit_back(i):
                # y matmuls + tanh + scores + exp for tile i
                xt8 = xt_tiles.pop(i)
                ypsA = ypsA_pool.tile([P, 512], F32, name="ypsA")
                ypsB = ypsB_pool.tile([P, 256], F32, name="ypsB")
                # fp8 DoubleRow: two 128-deep k-slabs per instruction.
                # All A-half matmuls first: the A accumulation stops two
                # matmuls earlier, so tanh-A (and the score chain behind
                # it) gets a head start on every tile.
                for jp in range(DJ // 2):
                    nc.tensor.matmul(
                        ypsA,
                        lhsT=xt8[:, 2 * jp : 2 * jp + 2, :],
                        rhs=w8[:, 2 * jp : 2 * jp + 2, 0:512],
                        start=(jp == 0),
                        stop=(jp == DJ // 2 - 1),
                        perf_mode=DR,
                    )
                for jp in range(DJ // 2):
                    nc.tensor.matmul(
                        ypsB,
                        lhsT=xt8[:, 2 * jp : 2 * jp + 2, :],
                        rhs=w8[:, 2 * jp : 2 * jp + 2, 512:D],
                        start=(jp == 0),
                        stop=(jp == DJ // 2 - 1),
                        perf_mode=DR,
                    )
                t = t_pool.tile([P, D], BF16, name="t")
                # y was computed against 16*W; tanh(y/16) undoes the scale
                nc.scalar.activation(
                    out=t[:, 0:512], in_=ypsA, func=ACTF.Tanh, scale=1.0 / 16
                )
                nc.scalar.activation(
                    out=t[:, 512:D], in_=ypsB, func=ACTF.Tanh, scale=1.0 / 16
                )
                if i % 4 == 0:
                    sc4 = sc_pool.tile([P, 4], F32, name="sc4")
                    sc_tiles[i // 4] = sc4
                sc4 = sc_tiles[i // 4]
                dve_out = t_pool.tile([P, D], FP8, name="dve_out")
                nc.vector.scalar_tensor_tensor(
                    out=dve_out,
                    in0=t,
                    scalar=1.0,
                    in1=v_bc,
                    op0=mybir.AluOpType.mult,
                    op1=mybir.AluOpType.mult,
                    accum_out=sc4[:, i % 4 : i % 4 + 1],
                )
                if i % 4 == 3:
                    g = i // 4
                    u4 = sc_pool.tile([P, 4], F32R, name="u4")
                    nc.scalar.activation(
                        out=u4, in_=sc_tiles.pop(g), func=ACTF.Exp,
                        accum_out=zg[:, g : g + 1],
                    )
                    u_tiles[g] = u4

            def emit_pool_tile(i, u):
                # f32r M=1 matmuls (1 cyc/row), one long accumulation
                # group in a single psum row across all 32 tiles
                xs = stage_tiles.pop(i)
                nc.tensor.matmul(
                    ppsA[0:1, :],
                    lhsT=u, rhs=xs[:, 0:512],
                    start=(i == 0), stop=(i == NT - 1),
                    skip_group_check=True,
                )
                nc.tensor.matmul(
                    ppsB[0:1, :],
                    lhsT=u, rhs=xs[:, 512:D],
                    start=(i == 0), stop=(i == NT - 1),
                    skip_group_check=True,
                )

            def emit_pool_group(k):
                u4 = u_tiles.pop(k // 4)
                for i in range(k - 3, k + 1):
                    emit_pool_tile(i, u4[:, i % 4 : i % 4 + 1])

            # W pair j is always emitted at least one iteration before the
            # first y-matmul that reads it (back runs 2 tiles behind front)
            emit_w_chunk(0)
            emit_w_chunk(1)
            emit_params()
            for i in range(NT + 11):
                if i < NT:
                    emit_front(i)
                if i == 0:
                    emit_w_chunk(2, eng=nc.sync)
                    emit_w_chunk(4)
                if i == 1:
                    emit_w_chunk(3, eng=nc.sync)
                    emit_w_chunk(5)
                # pool group k trails the y-chain of its tiles by ~9
                # iterations: pool groups are in-order PE barriers, so they
                # must never lead the ACT/DVE exp chain they depend on
                k = i - 11
                if k >= 3 and k % 4 == 3:
                    emit_pool_group(k)
                if i == 2:
                    nc.vector.tensor_copy(out=v_bc, in_=v_f32)
                if 2 <= i <= NT + 1:
                    emit_back(i - 2)

            # write out the unnormalized p row and the Z partials; the host
            # divides p by sum(out_z)
            p_sb = singles.tile([1, D], F32)
            nc.scalar.copy(out=p_sb[:, 0:512], in_=ppsA[0:1, :])
            nc.scalar.copy(out=p_sb[:, 512:D], in_=ppsB[0:1, :])
            nc.sync.dma_start(out=p_d[:, :], in_=p_sb)
            nc.sync.dma_start(out=z_d[:, :], in_=zg)

    if split_waits:
        _split_excess_waits(nc)
    return nc


def _split_excess_waits(nc: bass.Bass) -> None:
    """Walrus accepts a single HW sync-wait per instruction (EventSemaphore
    excepted). Tile can attach more (data dep + DMA-lane reuse). Move all but
    one wait onto InstEventSemaphore(s) inserted just before, on the same
    engine — the sequencer executes waits in order, so semantics are
    unchanged."""
    fn = nc.m.functions[0]
    for blk in fn.blocks:
        insts = blk.instructions
        new_insts = []
        for inst in insts:
            si = inst.sync_info
            if (
                not isinstance(inst, mybir.InstEventSemaphore)
                and si is not None
                and len(si.on_wait) > 1
            ):
                waits = list(si.on_wait)
                for w in waits[:-1]:
                    ev = mybir.InstEventSemaphore(
                        name=nc.get_next_instruction_name(), ins=[], outs=[]
                    )
                    ev.engine = inst.engine
                    ev.sync_info = mybir.SyncInfo(on_wait=[w], on_update=[])
                    new_insts.append(ev)
                inst.sync_info = mybir.SyncInfo(
                    on_wait=waits[-1:], on_update=list(si.on_update)
                )
            new_insts.append(inst)
        blk.instructions = new_insts


_CACHE: dict = {}
LAST_RESULT = None


def _get_nc() -> bass.Bass:
    if "nc" not in _CACHE:
        _CACHE["nc"] = _build()
    return _CACHE["nc"]


def kernel(x: np.ndarray, att_v: np.ndarray, att_W: np.ndarray) -> np.ndarray:
    global LAST_RESULT
    assert x.shape == (NCORES, S, D), x.shape
    nc = _get_nc()
    in_maps = [
        {
            "x": np.ascontiguousarray(x[b], dtype=np.float32),
            "att_v": np.ascontiguousarray(att_v, dtype=np.float32),
            "att_W": np.ascontiguousarray(att_W, dtype=np.float32),
        }
        for b in range(NCORES)
    ]
    res = run_bass_kernel_spmd(nc, in_maps, core_ids=list(range(NCORES)))
    LAST_RESULT = res
    outs = []
    for b in range(NCORES):
        p = res.results[b]["out_p"][0].astype(np.float64)
        z = res.results[b]["out_z"].sum(dtype=np.float64)
        outs.append(p / z)
    return np.stack(outs).astype(np.float32)


# revision 45
# speedup vs baseline: 1.1441x; 1.0064x over previous
"""AttentionPool Trainium2 kernel.

Problem: x[B=8, S=4096, D=768] f32; att_v[768]; att_W[768, 768].
  y = tanh(x @ W); scores = y . v; w = softmax(scores over S); out = w . x  -> [B, D]

Sharding: pure data-parallel over batch B — one batch per NeuronCore, 8 cores,
no collectives.

Per-core pipeline (batch b), per 128-row sequence tile i:
  1. HWDGE f32 load of x tile into a staging ring (full-rate, no cast);
     att_W / att_v load on the second (Activation) DGE queue.
  2. PE transpose-mode (f32r: 1.5 cyc/row): x_tile -> xT psum
  3. copy-cast psum f32 -> SBUF fp8(e4m3) xT; split DVE (chunks 0-3) /
     ACT (chunks 4-5) to balance the per-tile vector-engine load
  4. PE: y = xT.T @ (16*W) fp8 DoubleRow, psum f32; 3 k-pairs x {512, 256}
  5. ACT: t = tanh(y_psum / 16) -> bf16
  6. DVE: scores[:, i%4] = sum_e t*v  (scalar_tensor_tensor accum_out)
  7. ACT, per 4 tiles: u = exp(scores) -> f32r (no max-subtraction needed:
     |scores| < ~0.5), accum_out -> Z partial column
  8. PE, per 4 tiles, deferred: p += u_i.T @ x_stage_i — f32r M=1
     matmuls (1 cyc/row) accumulating into one psum row over all tiles
Host: out = p / Z  (Z = sum of the per-partition exp accums).

The y-chain (emit_back) runs 2 tiles behind the load/transpose front so
the staggered att_W chunk loads (second DGE queue) are always emitted
before the first y-matmul that reads them. Pool groups trail their tiles
by 11 iterations: they are in-order PE barriers gated on the
y->tanh->stt->exp chain, so with less slack the PE stalls every 4 tiles.
The six W casts alternate ACT/DVE so neither early queue eats them all.

PSUM budget (8 banks x 2KB): every psum pool is split at the 512-f32 bank
width; psum is per-buf bank-granular: yA(1)+yB(1)+xtA(2)+xtB(2)+ppA(1)+ppB(1)
= 8 banks. y psum is single-buffered: tanh overlaps the next tile's
transposes, which are emitted between consecutive y-groups.

Measured ~93.3-98.7us on HW (baseline 121.5us), rel err 5.96e-3 (gate 2e-2).
"""

import sys

sys.path.insert(0, "/opt/trn_rl_repo")

import numpy as np

import concourse.bass as bass
import concourse.mybir as mybir
import concourse.tile as tile
from concourse.bass_utils import run_bass_kernel_spmd
from concourse.masks import make_identity

P = 128
S = 4096
D = 768
NT = S // P  # 32 sequence tiles
DJ = D // P  # 6 contraction chunks
NCORES = 8

F32 = mybir.dt.float32
F32R = mybir.dt.float32r
BF16 = mybir.dt.bfloat16
FP8 = mybir.dt.float8e4
DR = mybir.MatmulPerfMode.DoubleRow
ACTF = mybir.ActivationFunctionType


def _build(split_waits: bool = True) -> bass.Bass:
    nc = bass.Bass()
    # x declared f32r (same bits as f32) so the f32r transpose/pool
    # matmuls see f32r-typed producers end-to-end (BIR verifier rule)
    x_d = nc.declare_dram_parameter("x", [S, D], F32R, isOutput=False)
    v_d = nc.declare_dram_parameter("att_v", [D], F32, isOutput=False)
    w_d = nc.declare_dram_parameter("att_W", [D, D], F32, isOutput=False)
    p_d = nc.declare_dram_parameter("out_p", [1, D], F32, isOutput=True)
    z_d = nc.declare_dram_parameter("out_z", [P, NT // 4], F32, isOutput=True)

    with tile.TileContext(nc) as tc:
        with (
            tc.tile_pool(name="singles", bufs=1) as singles,
            tc.tile_pool(name="stage", bufs=16) as stage_pool,
            tc.tile_pool(name="xt", bufs=6) as xt_pool,
            tc.tile_pool(name="tbuf", bufs=10) as t_pool,
            tc.tile_pool(name="sc", bufs=9) as sc_pool,
            tc.tile_pool(name="ypsA", bufs=1, space="PSUM") as ypsA_pool,
            tc.tile_pool(name="ypsB", bufs=1, space="PSUM") as ypsB_pool,
            tc.tile_pool(name="xtpA", bufs=2, space="PSUM") as xtpA_pool,
            tc.tile_pool(name="xtpB", bufs=2, space="PSUM") as xtpB_pool,
            tc.tile_pool(name="ppsA", bufs=1, space="PSUM") as ppsA_pool,
            tc.tile_pool(name="ppsB", bufs=1, space="PSUM") as ppsB_pool,
        ):
            # f32r identity for the f32r transposes: gpsimd can't write
            # f32r (ISA), so build in f32 and round via a scalar-engine copy
            ident_f32 = singles.tile([P, P], F32)
            make_identity(nc, ident_f32)
            ident = singles.tile([P, P], F32R)
            nc.scalar.copy(out=ident, in_=ident_f32)
            v_f32 = singles.tile([P, D], F32)
            v_bc = singles.tile([P, D], BF16)
            w_f32 = singles.tile([P, DJ, D], F32)
            w8 = singles.tile([P, DJ, D], FP8)
            # per-group partial Z accumulators; host sums the values.
            zg = singles.tile([P, NT // 4], F32)
            # pooling accumulator psum row (partition 0), one accumulation
            # group across all 32 tiles, split at the psum bank boundary
            ppsA = ppsA_pool.tile([P, 512], F32)
            ppsB = ppsB_pool.tile([P, 256], F32)

            stage_tiles = {}
            xt_tiles = {}
            u_tiles = {}
            sc_tiles = {}

            def emit_w_chunk(j, eng=None):
                # Both DGE queues: six serialized W loads on one queue land
                # the last chunk at ~6.6us, stalling y(0)'s last k-pair.
                (eng or nc.scalar).dma_start(
                    out=w_f32[:, j, :], in_=w_d[j * P : (j + 1) * P, :]
                )
                # fp8 e4m3 W scaled by 16 to keep small entries out of
                # the subnormal range; tanh() folds the 1/16 back in.
                # All six on ACT: a DMA-gated cast at the head of the
                # in-order DVE queue would block every copyA behind it.
                nc.scalar.activation(
                    out=w8[:, j, :], in_=w_f32[:, j, :],
                    func=ACTF.Copy, scale=16.0,
                )

            def emit_params():
                # v DMA lands between W1 and W4 on the scalar queue (~3.3us)
                # so the v_bc copy never blocks the early DVE queue
                nc.scalar.dma_start(
                    out=v_f32, in_=v_d[:][None, :].to_broadcast([P, D])
                )


            def emit_front(i):
                # load + f32r transpose + fp8 copy-out for tile i
                xs = stage_pool.tile([P, D], F32R, name="xs")
                nc.sync.dma_start(out=xs, in_=x_d[i * P : (i + 1) * P, :])
                stage_tiles[i] = xs
                xtA = xtpA_pool.tile([P, 512], F32, name="xtA")
                xtB = xtpB_pool.tile([P, 256], F32, name="xtB")
                for j in range(DJ):
                    dst = (
                        xtA[:, j * P : (j + 1) * P]
                        if j < 4
                        else xtB[:, (j - 4) * P : (j - 3) * P]
                    )
                    nc.tensor.transpose(
                        dst.bitcast(F32R),
                        xs[:, j * P : (j + 1) * P],
                        ident[:],
                    )
                xt8 = xt_pool.tile([P, DJ, P], FP8, name="xt8")
                nc.vector.tensor_copy(out=xt8[:, 0:4, :], in_=xtA)
                nc.scalar.copy(out=xt8[:, 4:6, :], in_=xtB)
                xt_tiles[i] = xt8

            def emit_back(i):
                # y matmuls + tanh + scores + exp for tile i
                xt8 = xt_tiles.pop(i)
                ypsA = ypsA_pool.tile([P, 512], F32, name="ypsA")
                ypsB = ypsB_pool.tile([P, 256], F32, name="ypsB")
                # fp8 DoubleRow: two 128-deep k-slabs per instruction.
                # All A-half matmuls first: the A accumulation stops two
                # matmuls earlier, so tanh-A (and the score chain behind
                # it) gets a head start on every tile.
                for jp in range(DJ // 2):
                    nc.tensor.matmul(
                        ypsA,
                        lhsT=xt8[:, 2 * jp : 2 * jp + 2, :],
                        rhs=w8[:, 2 * jp : 2 * jp + 2, 0:512],
                        start=(jp == 0),
                        stop=(jp == DJ // 2 - 1),
                        perf_mode=DR,
                    )
                for jp in range(DJ // 2):
                    nc.tensor.matmul(
                        ypsB,
                        lhsT=xt8[:, 2 * jp : 2 * jp + 2, :],
                        rhs=w8[:, 2 * jp : 2 * jp + 2, 512:D],
                        start=(jp == 0),
                        stop=(jp == DJ // 2 - 1),
                        perf_mode=DR,
                    )
                t = t_pool.tile([P, D], BF16, name="t")
                # y was computed against 16*W; tanh(y/16) undoes the scale
                nc.scalar.activation(
                    out=t[:, 0:512], in_=ypsA, func=ACTF.Tanh, scale=1.0 / 16
                )
                nc.scalar.activation(
                    out=t[:, 512:D], in_=ypsB, func=ACTF.Tanh, scale=1.0 / 16
                )
                if i % 4 == 0:
                    sc4 = sc_pool.tile([P, 4], F32, name="sc4")
                    sc_tiles[i // 4] = sc4
                sc4 = sc_tiles[i // 4]
                dve_out = t_pool.tile([P, D], FP8, name="dve_out")
                nc.vector.scalar_tensor_tensor(
                    out=dve_out,
                    in0=t,
                    scalar=1.0,
                    in1=v_bc,
                    op0=mybir.AluOpType.mult,
                    op1=mybir.AluOpType.mult,
                    accum_out=sc4[:, i % 4 : i % 4 + 1],
                )
                if i % 4 == 3:
                    g = i // 4
                    u4 = sc_pool.tile([P, 4], F32R, name="u4")
                    nc.scalar.activation(
                        out=u4, in_=sc_tiles.pop(g), func=ACTF.Exp,
                        accum_out=zg[:, g : g + 1],
                    )
                    u_tiles[g] = u4

            def emit_pool_tile(i, u):
                # f32r M=1 matmuls (1 cyc/row), one long accumulation
                # group in a single psum row across all 32 tiles
                xs = stage_tiles.pop(i)
                nc.tensor.matmul(
                    ppsA[0:1, :],
                    lhsT=u, rhs=xs[:, 0:512],
                    start=(i == 0), stop=(i == NT - 1),
                    skip_group_check=True,
                )
                nc.tensor.matmul(
                    ppsB[0:1, :],
                    lhsT=u, rhs=xs[:, 512:D],
                    start=(i == 0), stop=(i == NT - 1),
                    skip_group_check=True,
                )

            def emit_pool_group(k):
                u4 = u_tiles.pop(k // 4)
                for i in range(k - 3, k + 1):
                    emit_pool_tile(i, u4[:, i % 4 : i % 4 + 1])

            # W pair j is always emitted at least one iteration before the
            # first y-matmul that reads it (back runs 2 tiles behind front)
            emit_w_chunk(0)
            emit_w_chunk(1)
            emit_params()
            for i in range(NT + 11):
                if i < NT:
                    emit_front(i)
                if i == 0:
                    emit_w_chunk(2, eng=nc.sync)
                    emit_w_chunk(4)
                if i == 1:
                    emit_w_chunk(3, eng=nc.sync)
                    emit_w_chunk(5)
                # pool group k trails the y-chain of its tiles by ~9
                # iterations: pool groups are in-order PE barriers, so they
                # must never lead the ACT/DVE exp chain they depend on
                k = i - 11
                if k >= 3 and k % 4 == 3:
                    emit_pool_group(k)
                if i == 2:
                    nc.vector.tensor_copy(out=v_bc, in_=v_f32)
                if 2 <= i <= NT + 1:
                    emit_back(i - 2)

            # write out the unnormalized p row and the Z partials; the host
            # divides p by sum(out_z)
            p_sb = singles.tile([1, D], F32)
            nc.scalar.copy(out=p_sb[:, 0:512], in_=ppsA[0:1, :])
            nc.scalar.copy(out=p_sb[:, 512:D], in_=ppsB[0:1, :])
            nc.sync.dma_start(out=p_d[:, :], in_=p_sb)
            nc.sync.dma_start(out=z_d[:, :], in_=zg)

    if split_waits:
        _split_excess_waits(nc)
    return nc


def _split_excess_waits(nc: bass.Bass) -> None:
    """Walrus accepts a single HW sync-wait per instruction (EventSemaphore
    excepted). Tile can attach more (data dep + DMA-lane reuse). Move all but
    one wait onto InstEventSemaphore(s) inserted just before, on the same
    engine — the sequencer executes waits in order, so semantics are
    unchanged."""
    fn = nc.m.functions[0]
    for blk in fn.blocks:
        insts = blk.instructions
        new_insts = []
        for inst in insts:
            si = inst.sync_info
            if (
                not isinstance(inst, mybir.InstEventSemaphore)
                and si is not None
                and len(si.on_wait) > 1
            ):
                waits = list(si.on_wait)
                for w in waits[:-1]:
                    ev = mybir.InstEventSemaphore(
                        name=nc.get_next_instruction_name(), ins=[], outs=[]
                    )
                    ev.engine = inst.engine
                    ev.sync_info = mybir.SyncInfo(on_wait=[w], on_update=[])
                    new_insts.append(ev)
                inst.sync_info = mybir.SyncInfo(
                    on_wait=waits[-1:], on_update=list(si.on_update)
                )
            new_insts.append(inst)
        blk.instructions = new_insts


_CACHE: dict = {}
LAST_RESULT = None


def _get_nc() -> bass.Bass:
    if "nc" not in _CACHE:
        _CACHE["nc"] = _build()
    return _CACHE["nc"]


def kernel(x: np.ndarray, att_v: np.ndarray, att_W: np.ndarray) -> np.ndarray:
    global LAST_RESULT
    assert x.shape == (NCORES, S, D), x.shape
    nc = _get_nc()
    in_maps = [
        {
            "x": np.ascontiguousarray(x[b], dtype=np.float32),
            "att_v": np.ascontiguousarray(att_v, dtype=np.float32),
            "att_W": np.ascontiguousarray(att_W, dtype=np.float32),
        }
        for b in range(NCORES)
    ]
    res = run_bass_kernel_spmd(nc, in_maps, core_ids=list(range(NCORES)))
    LAST_RESULT = res
    outs = []
    for b in range(NCORES):
        p = res.results[b]["out_p"][0].astype(np.float64)
        z = res.results[b]["out_z"].sum(dtype=np.float64)
        outs.append(p / z)
    return np.stack(outs).astype(np.float32)


# revision 47
# speedup vs baseline: 1.1547x; 1.0092x over previous
"""AttentionPool Trainium2 kernel.

Problem: x[B=8, S=4096, D=768] f32; att_v[768]; att_W[768, 768].
  y = tanh(x @ W); scores = y . v; w = softmax(scores over S); out = w . x  -> [B, D]

Sharding: pure data-parallel over batch B — one batch per NeuronCore, 8 cores,
no collectives.

Per-core pipeline (batch b), per 128-row sequence tile i:
  1. HWDGE f32 load of x tile into a staging ring (full-rate, no cast);
     att_W / att_v load on the second (Activation) DGE queue.
  2. PE transpose-mode (f32r: 1.5 cyc/row): x_tile -> xT psum
  3. copy-cast psum f32 -> SBUF fp8(e4m3) xT; split DVE (chunks 0-3) /
     ACT (chunks 4-5) to balance the per-tile vector-engine load
  4. PE: y = xT.T @ (16*W) fp8 DoubleRow, psum f32; 3 k-pairs x {512, 256}
  5. ACT: t = tanh(y_psum / 16) -> bf16
  6. DVE: scores[:, i%4] = sum_e t*v  (scalar_tensor_tensor accum_out)
  7. ACT, per 4 tiles: u = exp(scores) -> f32r (no max-subtraction needed:
     |scores| < ~0.5), accum_out -> Z partial column
  8. PE, per 4 tiles, deferred: p += u_i.T @ x_stage_i — f32r M=1
     matmuls (1 cyc/row) accumulating into one psum row over all tiles
Host: out = p / Z  (Z = sum of the per-partition exp accums).

The y-chain (emit_back) runs 2 tiles behind the load/transpose front so
the staggered att_W chunk loads (second DGE queue) are always emitted
before the first y-matmul that reads them. Pool groups trail their tiles
by 11 iterations: they are in-order PE barriers gated on the
y->tanh->stt->exp chain, so with less slack the PE stalls every 4 tiles.
The six W casts alternate ACT/DVE so neither early queue eats them all.

PSUM budget (8 banks x 2KB): every psum pool is split at the 512-f32 bank
width; psum is per-buf bank-granular: yA(1)+yB(1)+xtA(2)+xtB(2)+ppA(1)+ppB(1)
= 8 banks. y psum is single-buffered: tanh overlaps the next tile's
transposes, which are emitted between consecutive y-groups.

Measured ~92.7-98.7us on HW (baseline 121.5us), rel err 5.96e-3 (gate 2e-2).
"""

import sys

sys.path.insert(0, "/opt/trn_rl_repo")

import numpy as np

import concourse.bass as bass
import concourse.mybir as mybir
import concourse.tile as tile
from concourse.bass_utils import run_bass_kernel_spmd
from concourse.masks import make_identity

P = 128
S = 4096
D = 768
NT = S // P  # 32 sequence tiles
DJ = D // P  # 6 contraction chunks
NCORES = 8

F32 = mybir.dt.float32
F32R = mybir.dt.float32r
BF16 = mybir.dt.bfloat16
FP8 = mybir.dt.float8e4
DR = mybir.MatmulPerfMode.DoubleRow
ACTF = mybir.ActivationFunctionType


def _build(split_waits: bool = True) -> bass.Bass:
    nc = bass.Bass()
    # x declared f32r (same bits as f32) so the f32r transpose/pool
    # matmuls see f32r-typed producers end-to-end (BIR verifier rule)
    x_d = nc.declare_dram_parameter("x", [S, D], F32R, isOutput=False)
    v_d = nc.declare_dram_parameter("att_v", [D], F32, isOutput=False)
    w_d = nc.declare_dram_parameter("att_W", [D, D], F32, isOutput=False)
    p_d = nc.declare_dram_parameter("out_p", [1, D], F32, isOutput=True)
    z_d = nc.declare_dram_parameter("out_z", [P, NT // 4], F32, isOutput=True)

    with tile.TileContext(nc) as tc:
        with (
            tc.tile_pool(name="singles", bufs=1) as singles,
            tc.tile_pool(name="stage", bufs=19) as stage_pool,
            tc.tile_pool(name="xt", bufs=8) as xt_pool,
            tc.tile_pool(name="tbuf", bufs=10) as t_pool,
            tc.tile_pool(name="sc", bufs=9) as sc_pool,
            tc.tile_pool(name="ypsA", bufs=1, space="PSUM") as ypsA_pool,
            tc.tile_pool(name="ypsB", bufs=1, space="PSUM") as ypsB_pool,
            tc.tile_pool(name="xtpA", bufs=2, space="PSUM") as xtpA_pool,
            tc.tile_pool(name="xtpB", bufs=2, space="PSUM") as xtpB_pool,
            tc.tile_pool(name="ppsA", bufs=1, space="PSUM") as ppsA_pool,
            tc.tile_pool(name="ppsB", bufs=1, space="PSUM") as ppsB_pool,
        ):
            # f32r identity for the f32r transposes: gpsimd can't write
            # f32r (ISA), so build in f32 and round via a scalar-engine copy
            ident_f32 = singles.tile([P, P], F32)
            make_identity(nc, ident_f32)
            ident = singles.tile([P, P], F32R)
            nc.scalar.copy(out=ident, in_=ident_f32)
            v_f32 = singles.tile([P, D], F32)
            v_bc = singles.tile([P, D], BF16)
            w_f32 = singles.tile([P, DJ, D], F32)
            w8 = singles.tile([P, DJ, D], FP8)
            # per-group partial Z accumulators; host sums the values.
            zg = singles.tile([P, NT // 4], F32)
            # pooling accumulator psum row (partition 0), one accumulation
            # group across all 32 tiles, split at the psum bank boundary
            ppsA = ppsA_pool.tile([P, 512], F32)
            ppsB = ppsB_pool.tile([P, 256], F32)

            stage_tiles = {}
            xt_tiles = {}
            u_tiles = {}
            sc_tiles = {}

            def emit_w_chunk(j, eng=None):
                # Both DGE queues: six serialized W loads on one queue land
                # the last chunk at ~6.6us, stalling y(0)'s last k-pair.
                (eng or nc.scalar).dma_start(
                    out=w_f32[:, j, :], in_=w_d[j * P : (j + 1) * P, :]
                )
                # fp8 e4m3 W scaled by 16 to keep small entries out of
                # the subnormal range; tanh() folds the 1/16 back in.
                # All six on ACT: a DMA-gated cast at the head of the
                # in-order DVE queue would block every copyA behind it.
                nc.scalar.activation(
                    out=w8[:, j, :], in_=w_f32[:, j, :],
                    func=ACTF.Copy, scale=16.0,
                )

            def emit_params():
                # v DMA lands between W1 and W4 on the scalar queue (~3.3us)
                # so the v_bc copy never blocks the early DVE queue
                nc.scalar.dma_start(
                    out=v_f32, in_=v_d[:][None, :].to_broadcast([P, D])
                )


            def emit_front(i):
                # load + f32r transpose + fp8 copy-out for tile i
                xs = stage_pool.tile([P, D], F32R, name="xs")
                nc.sync.dma_start(out=xs, in_=x_d[i * P : (i + 1) * P, :])
                stage_tiles[i] = xs
                xtA = xtpA_pool.tile([P, 512], F32, name="xtA")
                xtB = xtpB_pool.tile([P, 256], F32, name="xtB")
                for j in range(DJ):
                    dst = (
                        xtA[:, j * P : (j + 1) * P]
                        if j < 4
                        else xtB[:, (j - 4) * P : (j - 3) * P]
                    )
                    nc.tensor.transpose(
                        dst.bitcast(F32R),
                        xs[:, j * P : (j + 1) * P],
                        ident[:],
                    )
                xt8 = xt_pool.tile([P, DJ, P], FP8, name="xt8")
                nc.vector.tensor_copy(out=xt8[:, 0:4, :], in_=xtA)
                nc.scalar.copy(out=xt8[:, 4:6, :], in_=xtB)
                xt_tiles[i] = xt8

            def emit_back(i):
                # y matmuls + tanh + scores + exp for tile i
                xt8 = xt_tiles.pop(i)
                ypsA = ypsA_pool.tile([P, 512], F32, name="ypsA")
                ypsB = ypsB_pool.tile([P, 256], F32, name="ypsB")
                # fp8 DoubleRow: two 128-deep k-slabs per instruction.
                # All A-half matmuls first: the A accumulation stops two
                # matmuls earlier, so tanh-A (and the score chain behind
                # it) gets a head start on every tile.
                for jp in range(DJ // 2):
                    nc.tensor.matmul(
                        ypsA,
                        lhsT=xt8[:, 2 * jp : 2 * jp + 2, :],
                        rhs=w8[:, 2 * jp : 2 * jp + 2, 0:512],
                        start=(jp == 0),
                        stop=(jp == DJ // 2 - 1),
                        perf_mode=DR,
                    )
                for jp in range(DJ // 2):
                    nc.tensor.matmul(
                        ypsB,
                        lhsT=xt8[:, 2 * jp : 2 * jp + 2, :],
                        rhs=w8[:, 2 * jp : 2 * jp + 2, 512:D],
                        start=(jp == 0),
                        stop=(jp == DJ // 2 - 1),
                        perf_mode=DR,
                    )
                t = t_pool.tile([P, D], BF16, name="t")
                # y was computed against 16*W; tanh(y/16) undoes the scale
                nc.scalar.activation(
                    out=t[:, 0:512], in_=ypsA, func=ACTF.Tanh, scale=1.0 / 16
                )
                nc.scalar.activation(
                    out=t[:, 512:D], in_=ypsB, func=ACTF.Tanh, scale=1.0 / 16
                )
                if i % 4 == 0:
                    sc4 = sc_pool.tile([P, 4], F32, name="sc4")
                    sc_tiles[i // 4] = sc4
                sc4 = sc_tiles[i // 4]
                dve_out = t_pool.tile([P, D], FP8, name="dve_out")
                nc.vector.scalar_tensor_tensor(
                    out=dve_out,
                    in0=t,
                    scalar=1.0,
                    in1=v_bc,
                    op0=mybir.AluOpType.mult,
                    op1=mybir.AluOpType.mult,
                    accum_out=sc4[:, i % 4 : i % 4 + 1],
                )
                if i % 4 == 3:
                    g = i // 4
                    u4 = sc_pool.tile([P, 4], F32R, name="u4")
                    nc.scalar.activation(
                        out=u4, in_=sc_tiles.pop(g), func=ACTF.Exp,
                        accum_out=zg[:, g : g + 1],
                    )
                    u_tiles[g] = u4

            def emit_pool_tile(i, u):
                # f32r M=1 matmuls (1 cyc/row), one long accumulation
                # group in a single psum row across all 32 tiles
                xs = stage_tiles.pop(i)
                nc.tensor.matmul(
                    ppsA[0:1, :],
                    lhsT=u, rhs=xs[:, 0:512],
                    start=(i == 0), stop=(i == NT - 1),
                    skip_group_check=True,
                )
                nc.tensor.matmul(
                    ppsB[0:1, :],
                    lhsT=u, rhs=xs[:, 512:D],
                    start=(i == 0), stop=(i == NT - 1),
                    skip_group_check=True,
                )

            def emit_pool_group(k):
                u4 = u_tiles.pop(k // 4)
                for i in range(k - 3, k + 1):
                    emit_pool_tile(i, u4[:, i % 4 : i % 4 + 1])

            # W pair j is always emitted at least one iteration before the
            # first y-matmul that reads it (back runs 2 tiles behind front)
            emit_w_chunk(0)
            emit_w_chunk(1)
            emit_params()
            for i in range(NT + 11):
                if i < NT:
                    emit_front(i)
                if i == 0:
                    emit_w_chunk(2, eng=nc.sync)
                    emit_w_chunk(4)
                if i == 1:
                    emit_w_chunk(3, eng=nc.sync)
                    emit_w_chunk(5)
                # pool group k trails the y-chain of its tiles by ~9
                # iterations: pool groups are in-order PE barriers, so they
                # must never lead the ACT/DVE exp chain they depend on
                k = i - 11
                if k >= 3 and k % 4 == 3:
                    emit_pool_group(k)
                if i == 2:
                    nc.vector.tensor_copy(out=v_bc, in_=v_f32)
                if 2 <= i <= NT + 1:
                    emit_back(i - 2)

            # write out the unnormalized p row and the Z partials; the host
            # divides p by sum(out_z)
            p_sb = singles.tile([1, D], F32)
            nc.scalar.copy(out=p_sb[:, 0:512], in_=ppsA[0:1, :])
            nc.scalar.copy(out=p_sb[:, 512:D], in_=ppsB[0:1, :])
            nc.sync.dma_start(out=p_d[:, :], in_=p_sb)
            nc.sync.dma_start(out=z_d[:, :], in_=zg)

    if split_waits:
        _split_excess_waits(nc)
    return nc


def _split_excess_waits(nc: bass.Bass) -> None:
    """Walrus accepts a single HW sync-wait per instruction (EventSemaphore
    excepted). Tile can attach more (data dep + DMA-lane reuse). Move all but
    one wait onto InstEventSemaphore(s) inserted just before, on the same
    engine — the sequencer executes waits in order, so semantics are
    unchanged."""
    fn = nc.m.functions[0]
    for blk in fn.blocks:
        insts = blk.instructions
        new_insts = []
        for inst in insts:
            si = inst.sync_info
            if (
                not isinstance(inst, mybir.InstEventSemaphore)
                and si is not None
                and len(si.on_wait) > 1
            ):
                waits = list(si.on_wait)
                for w in waits[:-1]:
                    ev = mybir.InstEventSemaphore(
                        name=nc.get_next_instruction_name(), ins=[], outs=[]
                    )
                    ev.engine = inst.engine
                    ev.sync_info = mybir.SyncInfo(on_wait=[w], on_update=[])
                    new_insts.append(ev)
                inst.sync_info = mybir.SyncInfo(
                    on_wait=waits[-1:], on_update=list(si.on_update)
                )
            new_insts.append(inst)
        blk.instructions = new_insts


_CACHE: dict = {}
LAST_RESULT = None


def _get_nc() -> bass.Bass:
    if "nc" not in _CACHE:
        _CACHE["nc"] = _build()
    return _CACHE["nc"]


def kernel(x: np.ndarray, att_v: np.ndarray, att_W: np.ndarray) -> np.ndarray:
    global LAST_RESULT
    assert x.shape == (NCORES, S, D), x.shape
    nc = _get_nc()
    in_maps = [
        {
            "x": np.ascontiguousarray(x[b], dtype=np.float32),
            "att_v": np.ascontiguousarray(att_v, dtype=np.float32),
            "att_W": np.ascontiguousarray(att_W, dtype=np.float32),
        }
        for b in range(NCORES)
    ]
    res = run_bass_kernel_spmd(nc, in_maps, core_ids=list(range(NCORES)))
    LAST_RESULT = res
    outs = []
    for b in range(NCORES):
        p = res.results[b]["out_p"][0].astype(np.float64)
        z = res.results[b]["out_z"].sum(dtype=np.float64)
        outs.append(p / z)
    return np.stack(outs).astype(np.float32)
